# revision 16
# baseline (speedup 1.0000x reference)
"""Causal self-attention (B=2, T=4096, C=768, H=12, D=64) on 8 trn2 NeuronCores.

Sharding: (batch, head-group) — core c handles b = c//4 and heads 3*(c%4) .. 3*(c%4)+2.
Each core computes q/k/v for its 3 heads, the causal softmax attention matrix
(written as the `att` output), the attention-weighted values, and a partial
output projection (its heads' contribution to y). Host sums the 4 per-batch
partials and adds the bias.

Device kernel layout notes:
  - x is fed transposed (xT [C, T]) so all projections use it as the moving
    matmul operand with C on partitions.
  - qT/kT are produced directly in [D, T] layout (64 partitions); heads 0/1
    are stacked on partitions 0:64 / 64:128 of shared tiles, head 2 lives on
    partitions 0:64 of its own tile (matmul operands must share base
    partitions).
  - scores tiles [128 q, 512 k] -> masked (diag) -> exp on ACT with row-sum
    accumulation -> normalized in SBUF -> DMA'd to HBM (only the causal
    region; the upper triangle relies on zero-initialized output buffers)
    -> PE-transposed into [k, q] slabs -> att @ v accumulated in PSUM as
    yT [D, q] -> output projection.
  - matmul inputs use float32r (fast single-pass fp32) or plain float32
    (exact, 4x slower) depending on `r_mode`.
"""

import functools
import os

import numpy as np

import concourse.bass as bass
import concourse.mybir as mybir
import concourse.tile as tile
from concourse import bacc
from concourse.bass_utils import run_bass_kernel_spmd
from concourse.masks import make_identity

P = 128
T = 4096
C = 768
D = 64
HPC = 3          # heads per core
CT = C // P      # 6 C-tiles
FT = T // 512    # 8 free tiles over T
NQB = T // 256   # 16 q-blocks of 256 rows
NEG = -1.0e30
F32 = mybir.dt.float32
F32R = mybir.dt.float32r


def _build(r_mode: bool, reps: int = 1) -> bacc.Bacc:
    nc = bacc.Bacc("TRN2", target_bir_lowering=False, debug=False, num_devices=8)
    MM = F32R if r_mode else F32

    xT_d = nc.declare_dram_parameter("xT", [C, T], F32, isOutput=False)
    ws_d = nc.declare_dram_parameter("wstack", [C, 576], F32, isOutput=False)
    wp_d = nc.declare_dram_parameter("wpT", [192, C], F32, isOutput=False)
    att_d = nc.declare_dram_parameter("att", [HPC, T, T], F32, isOutput=True)
    y_d = nc.declare_dram_parameter("y", [T, C], F32, isOutput=True)

    xT_r = xT_d.ap().rearrange("(o p) t -> o p t", p=P)      # [6, 128, T]
    ws_r = ws_d.ap().rearrange("(o p) m -> p o m", p=P)      # [128, 6, 576]
    wp_r = wp_d.ap().rearrange("(h p) m -> p h m", p=D)      # [64, 3, C]

    trace_sim = os.environ.get("ATT_TRACE_SIM", "0") == "1"
    with tile.TileContext(nc, trace_sim=trace_sim) as tc:
        with (
            tc.tile_pool(name="const", bufs=1) as cpool,
            tc.tile_pool(name="qk", bufs=1) as qkpool,
            tc.tile_pool(name="vst", bufs=1) as vpool,
        ):
            # --- constants ---
            ident_f = cpool.tile([P, P], F32, tag="ident_f")
            make_identity(nc, ident_f[:])
            if r_mode:
                ident = cpool.tile([P, P], MM, tag="ident_r")
                nc.vector.tensor_copy(ident[:], ident_f[:])
            else:
                ident = ident_f
            zero_r = cpool.tile([P, 512], MM, tag="zero_r")
            nc.gpsimd.memset(zero_r.bitcast(F32), 0.0)
            # 4 causal bias masks [128, 512]: offset o keeps col <= row + o
            masks = []
            for mi in range(4):
                m = cpool.tile([P, 512], F32, tag=f"mask{mi}")
                nc.gpsimd.memset(m[:], 0.0)
                nc.gpsimd.affine_select(
                    out=m[:], in_=m[:],
                    compare_op=mybir.AluOpType.is_ge,
                    fill=NEG, base=mi * 128,
                    pattern=[[-1, 512]], channel_multiplier=1,
                )
                masks.append(m)
            # proj weights [64, 3, C] (persistent)
            wp_s = cpool.tile([D, HPC, C], MM, tag="wpT")

            # persistent per-head tensors
            # T1 = [q0|q1], T2 = [k0|k1] on partition halves; T3 = [q2, k2] planes
            T1 = qkpool.tile([P, T], MM, tag="T1")
            T2 = qkpool.tile([P, T], MM, tag="T2")
            T3 = qkpool.tile([D, 2, T], MM, tag="T3")
            vt = [
                vpool.tile([P, T // P, D], MM, tag=f"v{h}", name=f"v{h}")
                for h in range(HPC)
            ]

            q_ap = [T1[0:D], T1[D:P], T3[:, 0]]
            k_ap = [T2[0:D], T2[D:P], T3[:, 1]]

            for _rep in range(reps):
                # ---------------- phase 1: qkv projections ----------------
                with (
                    tc.tile_pool(name="xt", bufs=1) as xpool,
                    tc.tile_pool(name="wq", bufs=1) as wqpool,
                    tc.tile_pool(name="tmp", bufs=3) as tpool,
                    tc.tile_pool(name="ps1", bufs=3, space="PSUM") as ps1,
                    tc.tile_pool(name="ps1t", bufs=2, space="PSUM") as ps1t,
                ):
                    ws_s = wqpool.tile([P, CT, 576], MM, tag="wstack")
                    if r_mode:
                        ws_stg = wqpool.tile([P, CT, 576], F32, tag="wstack_stg")
                        nc.sync.dma_start(ws_stg[:], ws_r[:])
                        nc.vector.tensor_copy(ws_s[:], ws_stg[:])
                        wp_stg = wqpool.tile([D, HPC, C], F32, tag="wpT_stg")
                        nc.sync.dma_start(wp_stg[:], wp_r[:])
                        nc.vector.tensor_copy(wp_s[:], wp_stg[:])
                    else:
                        nc.sync.dma_start(ws_s[:], ws_r[:])
                        nc.sync.dma_start(wp_s[:], wp_r[:])

                    # matmul groups: (wstack col range, psum partitions)
                    groups = [
                        (0, 128),    # [q0|q1]
                        (128, 128),  # [k0|k1]
                        (256, 128),  # [v0|v1]
                        (384, 64),   # q2
                        (448, 64),   # k2
                        (512, 64),   # v2
                    ]
                    for quarter in range(4):
                        toff = quarter * 1024
                        xts = []
                        for ct in range(CT):
                            xt = xpool.tile([P, 1024], MM, tag=f"xt{ct}")
                            if r_mode:
                                xstg = tpool.tile(
                                    [P, 1024], F32, tag="xstg", name="xstg"
                                )
                                nc.sync.dma_start(
                                    xstg[:], xT_r[ct][:, toff:toff + 1024]
                                )
                                nc.vector.tensor_copy(xt[:], xstg[:])
                            else:
                                nc.sync.dma_start(
                                    xt[:], xT_r[ct][:, toff:toff + 1024]
                                )
                            xts.append(xt)
                        for lft in range(2):
                            fsl = slice(toff + lft * 512, toff + (lft + 1) * 512)
                            for gi, (coff, gp) in enumerate(groups):
                                gw = 128 if gp == 128 else 64
                                ps = ps1.tile([P, 512], F32, tag="qkv_ps")
                                for ct in range(CT):
                                    nc.tensor.matmul(
                                        ps[0:gp],
                                        ws_s[:, ct, coff:coff + gw],
                                        xts[ct][:, lft * 512:(lft + 1) * 512],
                                        start=(ct == 0), stop=(ct == CT - 1),
                                    )
                                if gi == 0:
                                    nc.any.tensor_copy(T1[:, fsl], ps[:])
                                elif gi == 1:
                                    nc.any.tensor_copy(T2[:, fsl], ps[:])
                                elif gi == 3:
                                    nc.any.tensor_copy(T3[:, 0, fsl], ps[0:D])
                                elif gi == 4:
                                    nc.any.tensor_copy(T3[:, 1, fsl], ps[0:D])
                                else:
                                    # v01 (gi==2) or v2 (gi==5): stage + transpose
                                    tmp = tpool.tile([P, 512], MM, tag="vtmp")
                                    if gi == 5:
                                        nc.vector.tensor_copy(tmp[D:P], zero_r[D:P])
                                    nc.any.tensor_copy(tmp[0:gp], ps[0:gp])
                                    for sub in range(4):
                                        kt = (fsl.start + sub * 128) // P
                                        pt = ps1t.tile([P, P], MM, tag="vT_ps")
                                        nc.tensor.transpose(
                                            pt[:],
                                            tmp[:, sub * 128:(sub + 1) * 128],
                                            ident[:],
                                        )
                                        if gi == 2:
                                            nc.any.tensor_copy(vt[0][:, kt, :], pt[:, 0:D])
                                            nc.any.tensor_copy(vt[1][:, kt, :], pt[:, D:P])
                                        else:
                                            nc.any.tensor_copy(vt[2][:, kt, :], pt[:, 0:D])

                # ---------------- phase 2: attention + projection ----------------
                with (
                    tc.tile_pool(name="exp", bufs=2) as epool,
                    tc.tile_pool(name="slab", bufs=3) as spool,
                    tc.tile_pool(name="yt", bufs=2) as ytpool,
                    tc.tile_pool(name="yo", bufs=2) as yopool,
                    tc.tile_pool(name="stat", bufs=4) as stpool,
                    tc.tile_pool(name="ps_s", bufs=2, space="PSUM") as ps_s,
                    tc.tile_pool(name="ps_t", bufs=2, space="PSUM") as ps_t,
                    tc.tile_pool(name="ps_y", bufs=2, space="PSUM") as ps_y,
                    tc.tile_pool(name="ps_p", bufs=2, space="PSUM") as ps_p,
                ):
                    for qb in range(NQB):
                        yT_sb = ytpool.tile([D, HPC, 256], MM, tag="yT")
                        for h in range(HPC):
                            ebuf = epool.tile([P, 2, T], MM, tag="ebuf")
                            sums = stpool.tile([P, 2, 8], F32, tag="sums")
                            for qc in range(2):
                                q0 = qb * 256 + qc * 128
                                gi = 2 * qb + qc
                                nkt = gi // 4 + 1
                                for kt in range(nkt):
                                    ps = ps_s.tile([P, 512], F32, tag="sc")
                                    nc.tensor.matmul(
                                        ps[:],
                                        q_ap[h][:, q0:q0 + 128],
                                        k_ap[h][:, kt * 512:(kt + 1) * 512],
                                        start=True, stop=True,
                                    )
                                    if kt == gi // 4:
                                        nc.vector.tensor_add(
                                            ps[:], ps[:], masks[gi % 4][:]
                                        )
                                    nc.scalar.activation(
                                        ebuf[:, qc, kt * 512:(kt + 1) * 512],
                                        ps[:],
                                        mybir.ActivationFunctionType.Exp,
                                        scale=0.125,
                                        accum_out=sums[:, qc, kt:kt + 1],
                                    )
                                ssum = stpool.tile([P, 1], F32, tag="ssum")
                                if nkt > 1:
                                    nc.vector.tensor_reduce(
                                        ssum[:], sums[:, qc, 0:nkt],
                                        mybir.AxisListType.X, mybir.AluOpType.add,
                                    )
                                else:
                                    nc.vector.tensor_copy(ssum[:], sums[:, qc, 0:1])
                                recip = stpool.tile([P, 1], F32, tag="recip")
                                nc.vector.reciprocal(recip[:], ssum[:])
                                row = ebuf[:, qc, 0:q0 + 128]
                                nc.vector.tensor_scalar_mul(row, row, recip[:])
                                nc.sync.dma_start(
                                    att_d.ap()[h, q0:q0 + 128, 0:q0 + 128],
                                    row.bitcast(F32),
                                )
                            # transposes + att @ v for the 256-row block
                            nk128 = 2 * qb + 2
                            ytp = ps_y.tile([D, 256], F32, tag="ytp")
                            for kt in range(nk128):
                                slab = spool.tile([P, 256], MM, tag="slab")
                                for qc in range(2):
                                    q0 = qb * 256 + qc * 128
                                    if kt * 128 <= q0 + 127:
                                        pt = ps_t.tile([P, P], MM, tag="attT")
                                        nc.tensor.transpose(
                                            pt[:],
                                            ebuf[:, qc, kt * 128:(kt + 1) * 128],
                                            ident[:],
                                        )
                                        nc.any.tensor_copy(
                                            slab[:, qc * 128:(qc + 1) * 128], pt[:]
                                        )
                                    else:
                                        nc.vector.tensor_copy(
                                            slab[:, qc * 128:(qc + 1) * 128],
                                            zero_r[:, 0:128],
                                        )
                                nc.tensor.matmul(
                                    ytp[:],
                                    vt[h][:, kt, :],
                                    slab[:],
                                    start=(kt == 0), stop=(kt == nk128 - 1),
                                )
                            nc.any.tensor_copy(yT_sb[:, h, :], ytp[:])
                        # output projection for this q-block
                        for qc in range(2):
                            q0 = qb * 256 + qc * 128
                            yo = yopool.tile([P, C], F32, tag="yo")
                            for ns, (n0, nw) in enumerate(((0, 512), (512, 256))):
                                pj = ps_p.tile([P, 512], F32, tag="pj")
                                for h in range(HPC):
                                    nc.tensor.matmul(
                                        pj[:, 0:nw],
                                        yT_sb[:, h, qc * 128:(qc + 1) * 128],
                                        wp_s[:, h, n0:n0 + nw],
                                        start=(h == 0), stop=(h == HPC - 1),
                                    )
                                nc.any.tensor_copy(yo[:, n0:n0 + nw], pj[:, 0:nw])
                            nc.sync.dma_start(y_d.ap()[q0:q0 + 128, :], yo[:])

    nc.finalize()
    return nc


@functools.lru_cache(maxsize=4)
def _get_nc(r_mode: bool, reps: int) -> bacc.Bacc:
    return _build(r_mode, reps)


def _in_maps(x, Wq, Wk, Wv, Wp):
    x = np.asarray(x, dtype=np.float32)
    Wq = np.asarray(Wq, dtype=np.float32)
    Wk = np.asarray(Wk, dtype=np.float32)
    Wv = np.asarray(Wv, dtype=np.float32)
    Wp = np.asarray(Wp, dtype=np.float32)
    xT = [np.ascontiguousarray(x[b].T) for b in range(2)]
    maps = []
    for c in range(8):
        b, g = divmod(c, 4)
        hs = slice(g * 192, (g + 1) * 192)
        wq = Wq[hs].T  # [768, 192]
        wk = Wk[hs].T
        wv = Wv[hs].T
        wstack = np.ascontiguousarray(
            np.concatenate(
                [wq[:, 0:128], wk[:, 0:128], wv[:, 0:128],
                 wq[:, 128:192], wk[:, 128:192], wv[:, 128:192]],
                axis=1,
            )
        )
        wpT = np.ascontiguousarray(Wp[:, hs].T)  # [192, 768]
        maps.append({"xT": xT[b], "wstack": wstack, "wpT": wpT})
    return maps


def kernel(x, Wq, Wk, Wv, Wp, bp):
    r_mode = os.environ.get("ATT_MM_DTYPE", "fp32r") != "fp32"
    reps = int(os.environ.get("ATT_REPS", "1"))
    nc = _get_nc(r_mode, reps)
    maps = _in_maps(x, Wq, Wk, Wv, Wp)
    res = run_bass_kernel_spmd(nc, maps, core_ids=list(range(8)))
    bp = np.asarray(bp, dtype=np.float32)
    y = np.empty((2, T, C), dtype=np.float32)
    att = np.empty((2, 12, T, T), dtype=np.float32)
    for b in range(2):
        acc = None
        for g in range(4):
            r = res.results[b * 4 + g]
            att[b, g * 3:(g + 1) * 3] = r["att"]
            acc = r["y"] if acc is None else acc + r["y"]
        y[b] = acc + bp
    return y, att


# revision 20
# speedup vs baseline: 1.8335x; 1.8335x over previous
"""Causal self-attention (B=2, T=4096, C=768, H=12, D=64) on 8 trn2 NeuronCores.

Sharding: (batch, head-group) — core c handles b = c//4 and heads 3*(c%4) .. 3*(c%4)+2.
Each core computes q/k/v for its 3 heads, the causal softmax attention matrix
(written as the `att` output), the attention-weighted values, and a partial
output projection (its heads' contribution to y). Host sums the 4 per-batch
partials and adds the bias.

Device kernel layout notes:
  - x is fed transposed (xT [C, T]) so all projections use it as the moving
    matmul operand with C on partitions.
  - qT/kT are produced directly in [D, T] layout (64 partitions); heads 0/1
    are stacked on partitions 0:64 / 64:128 of shared tiles, head 2 lives on
    partitions 0:64 of its own tile (matmul operands must share base
    partitions).
  - scores tiles [128 q, 512 k] -> masked (diag) -> exp on ACT with row-sum
    accumulation -> normalized in SBUF -> DMA'd to HBM (only the causal
    region; the upper triangle relies on zero-initialized output buffers)
    -> PE-transposed into [k, q] slabs -> att @ v accumulated in PSUM as
    yT [D, q] -> output projection.
  - matmul inputs use float32r (fast single-pass fp32) or plain float32
    (exact, 4x slower) depending on `r_mode`.
"""

import functools
import os

import numpy as np

import concourse.bass as bass
import concourse.mybir as mybir
import concourse.tile as tile
from concourse import bacc
from concourse.bass_utils import run_bass_kernel_spmd
from concourse.masks import make_identity

P = 128
T = 4096
C = 768
D = 64
HPC = 3          # heads per core
CT = C // P      # 6 C-tiles
FT = T // 512    # 8 free tiles over T
NQB = T // 256   # 16 q-blocks of 256 rows
NEG = -1.0e30
F32 = mybir.dt.float32
F32R = mybir.dt.float32r


def _build(r_mode: bool, reps: int = 1) -> bacc.Bacc:
    nc = bacc.Bacc("TRN2", target_bir_lowering=False, debug=False, num_devices=8)
    MM = F32R if r_mode else F32

    xT_d = nc.declare_dram_parameter("xT", [C, T], F32, isOutput=False)
    ws_d = nc.declare_dram_parameter("wstack", [C, 576], F32, isOutput=False)
    wp_d = nc.declare_dram_parameter("wpT", [192, C], F32, isOutput=False)
    att_d = nc.declare_dram_parameter("att", [HPC, T, T], F32, isOutput=True)
    y_d = nc.declare_dram_parameter("y", [T, C], F32, isOutput=True)

    xT_r = xT_d.ap().rearrange("(o p) t -> o p t", p=P)      # [6, 128, T]
    ws_r = ws_d.ap().rearrange("(o p) m -> p o m", p=P)      # [128, 6, 576]
    wp_r = wp_d.ap().rearrange("(h p) m -> p h m", p=D)      # [64, 3, C]

    trace_sim = os.environ.get("ATT_TRACE_SIM", "0") == "1"
    with tile.TileContext(nc, trace_sim=trace_sim) as tc:
        with (
            tc.tile_pool(name="const", bufs=1) as cpool,
            tc.tile_pool(name="qk", bufs=1) as qkpool,
            tc.tile_pool(name="vst", bufs=1) as vpool,
        ):
            # --- constants ---
            ident_f = cpool.tile([P, P], F32, tag="ident_f")
            make_identity(nc, ident_f[:])
            if r_mode:
                ident = cpool.tile([P, P], MM, tag="ident_r")
                nc.vector.tensor_copy(ident[:], ident_f[:])
            else:
                ident = ident_f
            zero_r = cpool.tile([P, 512], MM, tag="zero_r")
            nc.gpsimd.memset(zero_r.bitcast(F32), 0.0)
            # 4 causal bias masks [128, 512]: offset o keeps col <= row + o
            masks = []
            for mi in range(4):
                m = cpool.tile([P, 512], F32, tag=f"mask{mi}")
                nc.gpsimd.memset(m[:], 0.0)
                nc.gpsimd.affine_select(
                    out=m[:], in_=m[:],
                    compare_op=mybir.AluOpType.is_ge,
                    fill=NEG, base=mi * 128,
                    pattern=[[-1, 512]], channel_multiplier=1,
                )
                masks.append(m)
            # proj weights [64, 3, C] (persistent)
            wp_s = cpool.tile([D, HPC, C], MM, tag="wpT")

            # persistent per-head tensors
            # T1 = [q0|q1], T2 = [k0|k1] on partition halves; T3 = [q2, k2] planes
            T1 = qkpool.tile([P, T], MM, tag="T1")
            T2 = qkpool.tile([P, T], MM, tag="T2")
            T3 = qkpool.tile([D, 2, T], MM, tag="T3")
            vt = [
                vpool.tile([P, T // P, D], MM, tag=f"v{h}", name=f"v{h}")
                for h in range(HPC)
            ]

            q_ap = [T1[0:D], T1[D:P], T3[:, 0]]
            k_ap = [T2[0:D], T2[D:P], T3[:, 1]]

            for _rep in range(reps):
                # ---------------- phase 1: qkv projections ----------------
                with (
                    tc.tile_pool(name="xt", bufs=1) as xpool,
                    tc.tile_pool(name="wq", bufs=1) as wqpool,
                    tc.tile_pool(name="tmp", bufs=3) as tpool,
                    tc.tile_pool(name="ps1", bufs=3, space="PSUM") as ps1,
                    tc.tile_pool(name="ps1t", bufs=2, space="PSUM") as ps1t,
                ):
                    ws_s = wqpool.tile([P, CT, 576], MM, tag="wstack")
                    if r_mode:
                        ws_stg = wqpool.tile([P, CT, 576], F32, tag="wstack_stg")
                        nc.sync.dma_start(ws_stg[:], ws_r[:])
                        nc.vector.tensor_copy(ws_s[:], ws_stg[:])
                        wp_stg = wqpool.tile([D, HPC, C], F32, tag="wpT_stg")
                        nc.sync.dma_start(wp_stg[:], wp_r[:])
                        nc.vector.tensor_copy(wp_s[:], wp_stg[:])
                    else:
                        nc.sync.dma_start(ws_s[:], ws_r[:])
                        nc.sync.dma_start(wp_s[:], wp_r[:])

                    # matmul groups: (wstack col range, psum partitions)
                    groups = [
                        (0, 128),    # [q0|q1]
                        (128, 128),  # [k0|k1]
                        (256, 128),  # [v0|v1]
                        (384, 64),   # q2
                        (448, 64),   # k2
                        (512, 64),   # v2
                    ]
                    for quarter in range(4):
                        toff = quarter * 1024
                        xts = []
                        for ct in range(CT):
                            xt = xpool.tile([P, 1024], MM, tag=f"xt{ct}")
                            if r_mode:
                                xstg = tpool.tile(
                                    [P, 1024], F32, tag="xstg", name="xstg"
                                )
                                nc.sync.dma_start(
                                    xstg[:], xT_r[ct][:, toff:toff + 1024]
                                )
                                nc.vector.tensor_copy(xt[:], xstg[:])
                            else:
                                nc.sync.dma_start(
                                    xt[:], xT_r[ct][:, toff:toff + 1024]
                                )
                            xts.append(xt)
                        for lft in range(2):
                            fsl = slice(toff + lft * 512, toff + (lft + 1) * 512)
                            for gi, (coff, gp) in enumerate(groups):
                                gw = 128 if gp == 128 else 64
                                ps = ps1.tile([P, 512], F32, tag="qkv_ps")
                                for ct in range(CT):
                                    nc.tensor.matmul(
                                        ps[0:gp],
                                        ws_s[:, ct, coff:coff + gw],
                                        xts[ct][:, lft * 512:(lft + 1) * 512],
                                        start=(ct == 0), stop=(ct == CT - 1),
                                    )
                                if gi == 0:
                                    nc.any.tensor_copy(T1[:, fsl], ps[:])
                                elif gi == 1:
                                    nc.any.tensor_copy(T2[:, fsl], ps[:])
                                elif gi == 3:
                                    nc.any.tensor_copy(T3[:, 0, fsl], ps[0:D])
                                elif gi == 4:
                                    nc.any.tensor_copy(T3[:, 1, fsl], ps[0:D])
                                else:
                                    # v01 (gi==2) or v2 (gi==5): stage + transpose
                                    tmp = tpool.tile([P, 512], MM, tag="vtmp")
                                    if gi == 5:
                                        nc.vector.tensor_copy(tmp[D:P], zero_r[D:P])
                                    nc.any.tensor_copy(tmp[0:gp], ps[0:gp])
                                    for sub in range(4):
                                        kt = (fsl.start + sub * 128) // P
                                        pt = ps1t.tile([P, P], MM, tag="vT_ps")
                                        nc.tensor.transpose(
                                            pt[:],
                                            tmp[:, sub * 128:(sub + 1) * 128],
                                            ident[:],
                                        )
                                        if gi == 2:
                                            nc.any.tensor_copy(vt[0][:, kt, :], pt[:, 0:D])
                                            nc.any.tensor_copy(vt[1][:, kt, :], pt[:, D:P])
                                        else:
                                            nc.any.tensor_copy(vt[2][:, kt, :], pt[:, 0:D])

                # ---------------- phase 2: attention + projection ----------------
                with (
                    tc.tile_pool(name="exp", bufs=2) as epool,
                    tc.tile_pool(name="slab", bufs=3) as spool,
                    tc.tile_pool(name="yt", bufs=2) as ytpool,
                    tc.tile_pool(name="yo", bufs=2) as yopool,
                    tc.tile_pool(name="stat", bufs=4) as stpool,
                    tc.tile_pool(name="ps_s", bufs=2, space="PSUM") as ps_s,
                    tc.tile_pool(name="ps_t", bufs=2, space="PSUM") as ps_t,
                    tc.tile_pool(name="ps_y", bufs=2, space="PSUM") as ps_y,
                    tc.tile_pool(name="ps_p", bufs=2, space="PSUM") as ps_p,
                ):
                    for qb in range(NQB):
                        yT_sb = ytpool.tile([D, HPC, 256], MM, tag="yT")
                        for h in range(HPC):
                            ebuf = epool.tile([P, 2, T], MM, tag="ebuf")
                            sums = stpool.tile([P, 2, 8], F32, tag="sums")
                            for qc in range(2):
                                q0 = qb * 256 + qc * 128
                                gi = 2 * qb + qc
                                nkt = gi // 4 + 1
                                for kt in range(nkt):
                                    ps = ps_s.tile([P, 512], F32, tag="sc")
                                    nc.tensor.matmul(
                                        ps[:],
                                        q_ap[h][:, q0:q0 + 128],
                                        k_ap[h][:, kt * 512:(kt + 1) * 512],
                                        start=True, stop=True,
                                    )
                                    if kt == gi // 4:
                                        nc.vector.tensor_add(
                                            ps[:], ps[:], masks[gi % 4][:]
                                        )
                                    nc.scalar.activation(
                                        ebuf[:, qc, kt * 512:(kt + 1) * 512],
                                        ps[:],
                                        mybir.ActivationFunctionType.Exp,
                                        scale=0.125,
                                        accum_out=sums[:, qc, kt:kt + 1],
                                    )
                                ssum = stpool.tile([P, 1], F32, tag="ssum")
                                if nkt > 1:
                                    nc.vector.tensor_reduce(
                                        ssum[:], sums[:, qc, 0:nkt],
                                        mybir.AxisListType.X, mybir.AluOpType.add,
                                    )
                                else:
                                    nc.vector.tensor_copy(ssum[:], sums[:, qc, 0:1])
                                recip = stpool.tile([P, 1], F32, tag="recip")
                                nc.vector.reciprocal(recip[:], ssum[:])
                                row = ebuf[:, qc, 0:q0 + 128]
                                nc.scalar.activation(
                                    row, row,
                                    mybir.ActivationFunctionType.Copy,
                                    scale=recip[:],
                                )
                                nc.sync.dma_start(
                                    att_d.ap()[h, q0:q0 + 128, 0:q0 + 128],
                                    row.bitcast(F32),
                                )
                            # transposes + att @ v for the 256-row block
                            nk128 = 2 * qb + 2
                            ytp = ps_y.tile([D, 256], F32, tag="ytp")
                            for kt in range(nk128):
                                slab = spool.tile([P, 256], MM, tag="slab")
                                for qc in range(2):
                                    q0 = qb * 256 + qc * 128
                                    if kt * 128 <= q0 + 127:
                                        pt = ps_t.tile([P, P], MM, tag="attT")
                                        nc.tensor.transpose(
                                            pt[:],
                                            ebuf[:, qc, kt * 128:(kt + 1) * 128],
                                            ident[:],
                                        )
                                        nc.vector.tensor_copy(
                                            slab[:, qc * 128:(qc + 1) * 128],
                                            pt[:],
                                        )
                                    else:
                                        nc.vector.tensor_copy(
                                            slab[:, qc * 128:(qc + 1) * 128],
                                            zero_r[:, 0:128],
                                        )
                                nc.tensor.matmul(
                                    ytp[:],
                                    vt[h][:, kt, :],
                                    slab[:],
                                    start=(kt == 0), stop=(kt == nk128 - 1),
                                )
                            nc.vector.tensor_copy(yT_sb[:, h, :], ytp[:])
                        # output projection for this q-block
                        for qc in range(2):
                            q0 = qb * 256 + qc * 128
                            yo = yopool.tile([P, C], F32, tag="yo")
                            for ns, (n0, nw) in enumerate(((0, 512), (512, 256))):
                                pj = ps_p.tile([P, 512], F32, tag="pj")
                                for h in range(HPC):
                                    nc.tensor.matmul(
                                        pj[:, 0:nw],
                                        yT_sb[:, h, qc * 128:(qc + 1) * 128],
                                        wp_s[:, h, n0:n0 + nw],
                                        start=(h == 0), stop=(h == HPC - 1),
                                    )
                                nc.any.tensor_copy(yo[:, n0:n0 + nw], pj[:, 0:nw])
                            nc.sync.dma_start(y_d.ap()[q0:q0 + 128, :], yo[:])

    nc.finalize()
    return nc


@functools.lru_cache(maxsize=4)
def _get_nc(r_mode: bool, reps: int) -> bacc.Bacc:
    return _build(r_mode, reps)


def _in_maps(x, Wq, Wk, Wv, Wp):
    x = np.asarray(x, dtype=np.float32)
    Wq = np.asarray(Wq, dtype=np.float32)
    Wk = np.asarray(Wk, dtype=np.float32)
    Wv = np.asarray(Wv, dtype=np.float32)
    Wp = np.asarray(Wp, dtype=np.float32)
    xT = [np.ascontiguousarray(x[b].T) for b in range(2)]
    maps = []
    for c in range(8):
        b, g = divmod(c, 4)
        hs = slice(g * 192, (g + 1) * 192)
        wq = Wq[hs].T  # [768, 192]
        wk = Wk[hs].T
        wv = Wv[hs].T
        wstack = np.ascontiguousarray(
            np.concatenate(
                [wq[:, 0:128], wk[:, 0:128], wv[:, 0:128],
                 wq[:, 128:192], wk[:, 128:192], wv[:, 128:192]],
                axis=1,
            )
        )
        wpT = np.ascontiguousarray(Wp[:, hs].T)  # [192, 768]
        maps.append({"xT": xT[b], "wstack": wstack, "wpT": wpT})
    return maps


def kernel(x, Wq, Wk, Wv, Wp, bp):
    r_mode = os.environ.get("ATT_MM_DTYPE", "fp32r") != "fp32"
    reps = int(os.environ.get("ATT_REPS", "1"))
    nc = _get_nc(r_mode, reps)
    maps = _in_maps(x, Wq, Wk, Wv, Wp)
    res = run_bass_kernel_spmd(nc, maps, core_ids=list(range(8)))
    bp = np.asarray(bp, dtype=np.float32)
    y = np.empty((2, T, C), dtype=np.float32)
    att = np.empty((2, 12, T, T), dtype=np.float32)
    for b in range(2):
        acc = None
        for g in range(4):
            r = res.results[b * 4 + g]
            att[b, g * 3:(g + 1) * 3] = r["att"]
            acc = r["y"] if acc is None else acc + r["y"]
        y[b] = acc + bp
    return y, att
